# revision 4
# baseline (speedup 1.0000x reference)
"""EMA linear-recurrence kernel for TRN2, 8 cores. Even/odd PE-hybrid.

Splits each time chain into scan positions t=2j+1 (DVE scan over the
squared-coefficient recurrence st_j = a^2 st_{j-1} + D_j, HALF the columns)
and reconstruction positions t=2j, computed on the otherwise-idle PE as
  psum = diag(a*w*S*(1+a)/S_y) @ st_shifted + I @ d      (bf16, one-shot)
where st_shifted is the scan output tile with the chain initial state in
column 0. Scan-half outputs are rescaled on DVE via the 2x-ported
tensor_scalar (0.5 cyc/col); PE results leave PSUM through ACT copy-convert.
Both int8 output halves interleave into one [P, T] tile via stride-2 views,
so a single contiguous DMA per half ships them out.

Input quantization (host): D_j = round((a x_{2j} + x_{2j+1}) / (S (1+a)))
int8 - the (1+a) normalization keeps the recurrence-propagated quantization
error at S/2 in output units for every w. d = w x_{2j}/S_y in bf16 (one-shot,
no recurrence amplification). a and a^2 stay f32.
"""

import sys

sys.path.insert(0, "/opt/trn_rl_repo")

import numpy as np

B, T, C = 16, 8192, 256
N_CORES = 8
B_LOC = B // N_CORES
P = 128
G = C // P
NCHAIN = B_LOC * G
J = T // 2                    # 4096 scan positions per chain
SB = 2048                     # scan/rescale block (2 per chain)
PB = 512                      # PE/PSUM block
NCONST = 2 * G + NCHAIN

_compiled = None


def _build():
    import concourse.tile as tile
    from concourse import bacc, mybir
    from concourse.mybir import AluOpType

    nc = bacc.Bacc("TRN2", target_bir_lowering=False, debug=False,
                   num_devices=N_CORES)
    f32 = mybir.dt.float32
    i8 = mybir.dt.int8
    bf16 = mybir.dt.bfloat16

    d_ap = nc.dram_tensor("dd", [NCHAIN, P, J], i8, kind="ExternalInput").ap()
    e_ap = nc.dram_tensor("de", [NCHAIN, P, J], bf16,
                          kind="ExternalInput").ap()
    c_ap = nc.dram_tensor("consts", [P, NCONST], f32,
                          kind="ExternalInput").ap()
    m_ap = nc.dram_tensor("mm", [P, (G + 1) * P], bf16,
                          kind="ExternalInput").ap()
    y_ap = nc.dram_tensor("y", [NCHAIN, P, T], i8, kind="ExternalOutput").ap()

    with tile.TileContext(nc) as tc:
        with (
            tc.tile_pool(name="const", bufs=1) as cpool,
            tc.tile_pool(name="din", bufs=4) as dpool,
            tc.tile_pool(name="ein", bufs=4) as epool,
            tc.tile_pool(name="st", bufs=4) as spool,
            tc.tile_pool(name="yout", bufs=4) as ypool,
            tc.tile_pool(name="ps", bufs=6, space="PSUM") as pspool,
        ):
            dts, ets, yts, sts = {}, {}, {}, {}
            for c in range(NCHAIN):
                dt = dpool.tile([P, J], i8, tag="d")
                dts[c] = dt
                et = epool.tile([P, J], bf16, tag="e")
                ets[c] = et
                yt = ypool.tile([P, T], i8, tag="y")
                yts[c] = yt
                stt = spool.tile([P, J + 1], bf16, tag="st")
                sts[c] = stt

            nc.sync.dma_start(dts[0][:, 0:1024], d_ap[0][:, 0:1024])
            c_t = cpool.tile([P, NCONST], f32)
            nc.sync.dma_start(c_t[:], c_ap[:])
            m_t = cpool.tile([P, (G + 1) * P], bf16)
            nc.sync.dma_start(m_t[:], m_ap[:])
            nc.sync.dma_start(dts[0][:, 1024:], d_ap[0][:, 1024:])
            for c in range(1, NCHAIN):
                nc.sync.dma_start(dts[c][:], d_ap[c])
            for c in range(NCHAIN):
                nc.sync.dma_start(ets[c][:], e_ap[c])
            a2_t = c_t[:, 0:G]
            gv_t = c_t[:, G:2 * G]
            y0_t = c_t[:, 2 * G:]
            ident = m_t[:, G * P:]

            # chain initial state into the shift column of each st tile
            for c in range(NCHAIN):
                nc.scalar.copy(sts[c][:, 0:1], y0_t[:, c:c + 1])

            carries = {c: y0_t[:, c:c + 1] for c in range(NCHAIN)}

            def scan_block(c, j0, n):
                g = c % G
                nc.vector.tensor_tensor_scan(
                    sts[c][:, 1 + j0:1 + j0 + n],
                    a2_t[:, g:g + 1].broadcast_to([P, n]),
                    dts[c][:, j0:j0 + n],
                    initial=carries[c],
                    op0=AluOpType.mult,
                    op1=AluOpType.add,
                )
                carries[c] = sts[c][:, j0 + n:j0 + n + 1]

            def resc_block(c, j0, n, on_act=False):
                g = c % G
                yv = yts[c][:].rearrange("p (j two) -> p j two", two=2)
                if on_act:
                    # ACT is idle early; offloading the first rescales
                    # shortens the packed DVE stream that gates the tail
                    nc.scalar.mul(yv[:, j0:j0 + n, 1],
                                  sts[c][:, 1 + j0:1 + j0 + n],
                                  gv_t[:, g:g + 1])
                else:
                    nc.vector.tensor_scalar(
                        yv[:, j0:j0 + n, 1], sts[c][:, 1 + j0:1 + j0 + n],
                        gv_t[:, g:g + 1], None, AluOpType.mult)

            def pe_block(c, j0, egress_dve=False):
                g = c % G
                ps = pspool.tile([P, PB], f32, tag="ps")
                nc.tensor.matmul(ps[:], m_t[:, g * P:(g + 1) * P],
                                 sts[c][:, j0:j0 + PB],
                                 start=True, stop=False)
                nc.tensor.matmul(ps[:], ident,
                                 ets[c][:, j0:j0 + PB],
                                 start=False, stop=True)
                yv = yts[c][:].rearrange("p (j two) -> p j two", two=2)
                if egress_dve:
                    # drain the last PSUM blocks on DVE once its scan
                    # stream is done - halves the serialized egress tail
                    nc.vector.tensor_scalar(
                        yv[:, j0:j0 + PB, 0], ps[:], 1.0, None,
                        AluOpType.mult)
                else:
                    nc.scalar.copy(yv[:, j0:j0 + PB, 0], ps[:])

            for pair in ((0, 1), (2, 3)):
                for rnd in range(2):
                    if pair == (0, 1) and rnd == 0:
                        scan_block(0, 0, 1024)
                        scan_block(0, 1024, 1024)
                        scan_block(1, 0, SB)
                    else:
                        for c in pair:
                            scan_block(c, rnd * SB, SB)
                    # chain 3 first in the final round: its egress stays on
                    # ACT while chain 2's drains on the freed-up DVE
                    last = pair == (2, 3) and rnd == 1
                    chains = (3, 2) if last else pair
                    for c in chains:
                        resc_block(c, rnd * SB, SB,
                                   on_act=(pair == (0, 1) and rnd == 0))
                        for k in range(SB // PB):
                            j0 = rnd * SB + k * PB
                            pe_block(c, j0, egress_dve=(last and c == 2))
                            if rnd == 1 and j0 + PB == 3 * J // 4:
                                nc.sync.dma_start(
                                    y_ap[c][:, T // 2:3 * T // 4],
                                    yts[c][:, T // 2:3 * T // 4])
                    for c in pair:
                        if rnd == 0:
                            nc.scalar.dma_start(
                                y_ap[c][:, :T // 2], yts[c][:, :T // 2])
                        else:
                            nc.sync.dma_start(
                                y_ap[c][:, 3 * T // 4:],
                                yts[c][:, 3 * T // 4:])

    nc.compile()
    return nc


def _get_compiled():
    global _compiled
    if _compiled is None:
        _compiled = _build()
    return _compiled


def _in_maps(inputs, initial_state, smooth):
    import ml_dtypes

    inputs = np.ascontiguousarray(inputs, dtype=np.float32)
    initial_state = np.ascontiguousarray(initial_state, dtype=np.float32)
    smooth = np.ascontiguousarray(smooth, dtype=np.float32)

    w = np.clip(smooth, 0.0, 1.0)
    a = (1.0 - w).astype(np.float32)

    S = float(np.abs(inputs).max()) / 126.5
    Sy = max(float(np.abs(inputs).max()),
             float(np.abs(initial_state).max())) / 126.5
    ws = np.where(w > 0.0, w, 1.0)

    # [core, b, j, par, g, p]; t = 2j + par
    xr = inputs.reshape(N_CORES, B_LOC, J, 2, G, P)
    x_ev = xr[:, :, :, 0]                     # t = 2j   [core, b, j, g, p]
    x_od = xr[:, :, :, 1]                     # t = 2j+1

    D = np.round((a.reshape(G, P) * x_ev + x_od)
                 / (S * (1.0 + a.reshape(G, P)))).astype(np.int8)
    dq = D.transpose(0, 1, 3, 4, 2).reshape(N_CORES, NCHAIN, P, J)

    dm = (w.reshape(G, P) * x_ev / Sy).astype(ml_dtypes.bfloat16)
    de = dm.transpose(0, 1, 3, 4, 2).reshape(N_CORES, NCHAIN, P, J)

    aG = a.reshape(G, P).T                    # [P, G]
    wG = w.reshape(G, P).T
    a2_pg = (aG * aG).astype(np.float32)
    gv_pg = (wG * S * (1.0 + aG) / Sy).astype(np.float32)
    init_all = initial_state / (ws * S * (1.0 + a))

    m_ch = (aG * wG * S * (1.0 + aG) / Sy).astype(np.float32)  # [P, G]
    mm = np.zeros((P, (G + 1) * P), dtype=ml_dtypes.bfloat16)
    for g in range(G):
        mm[:, g * P:(g + 1) * P][np.arange(P), np.arange(P)] = \
            m_ch[:, g].astype(ml_dtypes.bfloat16)
    mm[:, G * P:][np.arange(P), np.arange(P)] = 1.0

    in_maps = []
    for core in range(N_CORES):
        consts = np.empty((P, NCONST), dtype=np.float32)
        consts[:, 0:G] = a2_pg
        consts[:, G:2 * G] = gv_pg
        for b in range(B_LOC):
            for g in range(G):
                consts[:, 2 * G + b * G + g] = init_all[core * B_LOC + b,
                                                        g * P:(g + 1) * P]
        in_maps.append({
            "dd": np.ascontiguousarray(dq[core]),
            "de": np.ascontiguousarray(de[core]),
            "consts": consts,
            "mm": mm,
        })
    return in_maps, S, Sy, w


def kernel(inputs, initial_state, smooth):
    from concourse.bass_utils import run_bass_kernel_spmd

    nc = _get_compiled()
    in_maps, S, Sy, w = _in_maps(inputs, initial_state, smooth)
    res = run_bass_kernel_spmd(nc, in_maps, list(range(N_CORES)))

    yh = np.stack([res.results[c]["y"] for c in range(N_CORES)])
    out = (
        (yh.astype(np.float32) * Sy)
        .reshape(N_CORES, B_LOC, G, P, T)
        .transpose(0, 1, 4, 2, 3)
        .reshape(B, T, C)
    )
    zero = np.clip(np.asarray(smooth, dtype=np.float32), 0.0, 1.0) == 0.0
    if zero.any():
        out[:, :, zero] = np.asarray(initial_state,
                                     dtype=np.float32)[:, None, zero]
    return out


# revision 5
# speedup vs baseline: 1.0712x; 1.0712x over previous
"""EMA linear-recurrence kernel for TRN2, 8 cores. Even/odd PE-hybrid.

Splits each time chain into scan positions t=2j+1 (DVE scan over the
squared-coefficient recurrence st_j = a^2 st_{j-1} + D_j, HALF the columns)
and reconstruction positions t=2j, computed on the otherwise-idle PE as
  psum = diag(a*w*S*(1+a)/S_y) @ st_shifted + I @ d      (bf16, one-shot)
where st_shifted is the scan output tile with the chain initial state in
column 0. Scan-half outputs are rescaled on DVE via the 2x-ported
tensor_scalar (0.5 cyc/col); PE results leave PSUM through ACT copy-convert.
Both int8 output halves interleave into one [P, T] tile via stride-2 views,
so a single contiguous DMA per half ships them out.

Input quantization (host): D_j = round((a x_{2j} + x_{2j+1}) / (S (1+a)))
int8 - the (1+a) normalization keeps the recurrence-propagated quantization
error at S/2 in output units for every w. d = w x_{2j}/S_y in bf16 (one-shot,
no recurrence amplification). a and a^2 stay f32.
"""

import sys

sys.path.insert(0, "/opt/trn_rl_repo")

import numpy as np

B, T, C = 16, 8192, 256
N_CORES = 8
B_LOC = B // N_CORES
P = 128
G = C // P
NCHAIN = B_LOC * G
J = T // 2                    # 4096 scan positions per chain
SB = 2048                     # scan/rescale block (2 per chain)
PB = 512                      # PE/PSUM block
NCONST = 2 * G + NCHAIN

_compiled = None


def _build():
    import concourse.tile as tile
    from concourse import bacc, mybir
    from concourse.mybir import AluOpType

    nc = bacc.Bacc("TRN2", target_bir_lowering=False, debug=False,
                   num_devices=N_CORES)
    f32 = mybir.dt.float32
    i8 = mybir.dt.int8
    bf16 = mybir.dt.bfloat16

    d_ap = nc.dram_tensor("dd", [NCHAIN, P, J], i8, kind="ExternalInput").ap()
    e_ap = nc.dram_tensor("de", [NCHAIN, P, J], bf16,
                          kind="ExternalInput").ap()
    c_ap = nc.dram_tensor("consts", [P, NCONST], f32,
                          kind="ExternalInput").ap()
    m_ap = nc.dram_tensor("mm", [P, (G + 1) * P], bf16,
                          kind="ExternalInput").ap()
    y_ap = nc.dram_tensor("y", [NCHAIN, P, T], i8, kind="ExternalOutput").ap()

    with tile.TileContext(nc) as tc:
        with (
            tc.tile_pool(name="const", bufs=1) as cpool,
            tc.tile_pool(name="din", bufs=4) as dpool,
            tc.tile_pool(name="ein", bufs=4) as epool,
            tc.tile_pool(name="st", bufs=4) as spool,
            tc.tile_pool(name="yout", bufs=4) as ypool,
            tc.tile_pool(name="ps", bufs=6, space="PSUM") as pspool,
        ):
            dts, ets, yts, sts = {}, {}, {}, {}
            for c in range(NCHAIN):
                dt = dpool.tile([P, J], i8, tag="d")
                dts[c] = dt
                et = epool.tile([P, J], bf16, tag="e")
                ets[c] = et
                yt = ypool.tile([P, T], i8, tag="y")
                yts[c] = yt
                stt = spool.tile([P, J + 1], bf16, tag="st")
                sts[c] = stt

            nc.sync.dma_start(dts[0][:, 0:1024], d_ap[0][:, 0:1024])
            # consts ride the ACT HWDGE ring: the SP ring streams the x
            # inputs back-to-back without per-issue gaps ahead of them
            c_t = cpool.tile([P, NCONST], f32)
            nc.scalar.dma_start(c_t[:], c_ap[:])
            m_t = cpool.tile([P, (G + 1) * P], bf16)
            nc.scalar.dma_start(m_t[:], m_ap[:])
            nc.sync.dma_start(dts[0][:, 1024:], d_ap[0][:, 1024:])
            for c in range(1, NCHAIN):
                nc.sync.dma_start(dts[c][:], d_ap[c])
            for c in range(NCHAIN):
                nc.sync.dma_start(ets[c][:], e_ap[c])
            a2_t = c_t[:, 0:G]
            gv_t = c_t[:, G:2 * G]
            y0_t = c_t[:, 2 * G:]
            ident = m_t[:, G * P:]

            # chain initial state into the shift column of each st tile
            for c in range(NCHAIN):
                nc.scalar.copy(sts[c][:, 0:1], y0_t[:, c:c + 1])

            carries = {c: y0_t[:, c:c + 1] for c in range(NCHAIN)}

            def scan_block(c, j0, n):
                g = c % G
                nc.vector.tensor_tensor_scan(
                    sts[c][:, 1 + j0:1 + j0 + n],
                    a2_t[:, g:g + 1].broadcast_to([P, n]),
                    dts[c][:, j0:j0 + n],
                    initial=carries[c],
                    op0=AluOpType.mult,
                    op1=AluOpType.add,
                )
                carries[c] = sts[c][:, j0 + n:j0 + n + 1]

            def resc_block(c, j0, n, on_act=False):
                g = c % G
                yv = yts[c][:].rearrange("p (j two) -> p j two", two=2)
                if on_act:
                    # ACT is idle early; offloading the first rescales
                    # shortens the packed DVE stream that gates the tail
                    nc.scalar.mul(yv[:, j0:j0 + n, 1],
                                  sts[c][:, 1 + j0:1 + j0 + n],
                                  gv_t[:, g:g + 1])
                else:
                    nc.vector.tensor_scalar(
                        yv[:, j0:j0 + n, 1], sts[c][:, 1 + j0:1 + j0 + n],
                        gv_t[:, g:g + 1], None, AluOpType.mult)

            def pe_block(c, j0, egress_dve=False):
                g = c % G
                ps = pspool.tile([P, PB], f32, tag="ps")
                nc.tensor.matmul(ps[:], m_t[:, g * P:(g + 1) * P],
                                 sts[c][:, j0:j0 + PB],
                                 start=True, stop=False)
                nc.tensor.matmul(ps[:], ident,
                                 ets[c][:, j0:j0 + PB],
                                 start=False, stop=True)
                yv = yts[c][:].rearrange("p (j two) -> p j two", two=2)
                if egress_dve:
                    # drain the last PSUM blocks on DVE once its scan
                    # stream is done - halves the serialized egress tail
                    nc.vector.tensor_scalar(
                        yv[:, j0:j0 + PB, 0], ps[:], 1.0, None,
                        AluOpType.mult)
                else:
                    nc.scalar.copy(yv[:, j0:j0 + PB, 0], ps[:])

            for pair in ((0, 1), (2, 3)):
                for rnd in range(2):
                    if pair == (0, 1) and rnd == 0:
                        scan_block(0, 0, 1024)
                        scan_block(0, 1024, 1024)
                        scan_block(1, 0, SB)
                    else:
                        for c in pair:
                            scan_block(c, rnd * SB, SB)
                    # chain 3 first in the final round: its egress stays on
                    # ACT while chain 2's drains on the freed-up DVE
                    last = pair == (2, 3) and rnd == 1
                    chains = (3, 2) if last else pair
                    for c in chains:
                        resc_block(c, rnd * SB, SB,
                                   on_act=(pair == (0, 1) and rnd == 0))
                        for k in range(SB // PB):
                            j0 = rnd * SB + k * PB
                            pe_block(c, j0, egress_dve=(last and c == 2))
                            if rnd == 1 and j0 + PB == 3 * J // 4:
                                nc.sync.dma_start(
                                    y_ap[c][:, T // 2:3 * T // 4],
                                    yts[c][:, T // 2:3 * T // 4])
                    for c in pair:
                        if rnd == 0:
                            nc.scalar.dma_start(
                                y_ap[c][:, :T // 2], yts[c][:, :T // 2])
                        else:
                            nc.sync.dma_start(
                                y_ap[c][:, 3 * T // 4:],
                                yts[c][:, 3 * T // 4:])

    nc.compile()
    return nc


def _get_compiled():
    global _compiled
    if _compiled is None:
        _compiled = _build()
    return _compiled


def _in_maps(inputs, initial_state, smooth):
    import ml_dtypes

    inputs = np.ascontiguousarray(inputs, dtype=np.float32)
    initial_state = np.ascontiguousarray(initial_state, dtype=np.float32)
    smooth = np.ascontiguousarray(smooth, dtype=np.float32)

    w = np.clip(smooth, 0.0, 1.0)
    a = (1.0 - w).astype(np.float32)

    S = float(np.abs(inputs).max()) / 126.5
    Sy = max(float(np.abs(inputs).max()),
             float(np.abs(initial_state).max())) / 126.5
    ws = np.where(w > 0.0, w, 1.0)

    # [core, b, j, par, g, p]; t = 2j + par
    xr = inputs.reshape(N_CORES, B_LOC, J, 2, G, P)
    x_ev = xr[:, :, :, 0]                     # t = 2j   [core, b, j, g, p]
    x_od = xr[:, :, :, 1]                     # t = 2j+1

    D = np.round((a.reshape(G, P) * x_ev + x_od)
                 / (S * (1.0 + a.reshape(G, P)))).astype(np.int8)
    dq = D.transpose(0, 1, 3, 4, 2).reshape(N_CORES, NCHAIN, P, J)

    dm = (w.reshape(G, P) * x_ev / Sy).astype(ml_dtypes.bfloat16)
    de = dm.transpose(0, 1, 3, 4, 2).reshape(N_CORES, NCHAIN, P, J)

    aG = a.reshape(G, P).T                    # [P, G]
    wG = w.reshape(G, P).T
    a2_pg = (aG * aG).astype(np.float32)
    gv_pg = (wG * S * (1.0 + aG) / Sy).astype(np.float32)
    init_all = initial_state / (ws * S * (1.0 + a))

    m_ch = (aG * wG * S * (1.0 + aG) / Sy).astype(np.float32)  # [P, G]
    mm = np.zeros((P, (G + 1) * P), dtype=ml_dtypes.bfloat16)
    for g in range(G):
        mm[:, g * P:(g + 1) * P][np.arange(P), np.arange(P)] = \
            m_ch[:, g].astype(ml_dtypes.bfloat16)
    mm[:, G * P:][np.arange(P), np.arange(P)] = 1.0

    in_maps = []
    for core in range(N_CORES):
        consts = np.empty((P, NCONST), dtype=np.float32)
        consts[:, 0:G] = a2_pg
        consts[:, G:2 * G] = gv_pg
        for b in range(B_LOC):
            for g in range(G):
                consts[:, 2 * G + b * G + g] = init_all[core * B_LOC + b,
                                                        g * P:(g + 1) * P]
        in_maps.append({
            "dd": np.ascontiguousarray(dq[core]),
            "de": np.ascontiguousarray(de[core]),
            "consts": consts,
            "mm": mm,
        })
    return in_maps, S, Sy, w


def kernel(inputs, initial_state, smooth):
    from concourse.bass_utils import run_bass_kernel_spmd

    nc = _get_compiled()
    in_maps, S, Sy, w = _in_maps(inputs, initial_state, smooth)
    res = run_bass_kernel_spmd(nc, in_maps, list(range(N_CORES)))

    yh = np.stack([res.results[c]["y"] for c in range(N_CORES)])
    out = (
        (yh.astype(np.float32) * Sy)
        .reshape(N_CORES, B_LOC, G, P, T)
        .transpose(0, 1, 4, 2, 3)
        .reshape(B, T, C)
    )
    zero = np.clip(np.asarray(smooth, dtype=np.float32), 0.0, 1.0) == 0.0
    if zero.any():
        out[:, :, zero] = np.asarray(initial_state,
                                     dtype=np.float32)[:, None, zero]
    return out


# revision 6
# speedup vs baseline: 1.0840x; 1.0119x over previous
"""EMA linear-recurrence kernel for TRN2, 8 cores. Even/odd PE-hybrid.

Splits each time chain into scan positions t=2j+1 (DVE scan over the
squared-coefficient recurrence st_j = a^2 st_{j-1} + D_j, HALF the columns)
and reconstruction positions t=2j, computed on the otherwise-idle PE as
  psum = diag(a*w*S*(1+a)/S_y) @ st_shifted + I @ d      (bf16, one-shot)
where st_shifted is the scan output tile with the chain initial state in
column 0. Scan-half outputs are rescaled on DVE via the 2x-ported
tensor_scalar (0.5 cyc/col); PE results leave PSUM through ACT copy-convert.
Both int8 output halves interleave into one [P, T] tile via stride-2 views,
so a single contiguous DMA per half ships them out.

Input quantization (host): D_j = round((a x_{2j} + x_{2j+1}) / (S (1+a)))
int8 - the (1+a) normalization keeps the recurrence-propagated quantization
error at S/2 in output units for every w. d = w x_{2j}/S_y in bf16 (one-shot,
no recurrence amplification). a and a^2 stay f32.
"""

import sys

sys.path.insert(0, "/opt/trn_rl_repo")

import numpy as np

B, T, C = 16, 8192, 256
N_CORES = 8
B_LOC = B // N_CORES
P = 128
G = C // P
NCHAIN = B_LOC * G
J = T // 2                    # 4096 scan positions per chain
SB = 2048                     # scan/rescale block (2 per chain)
PB = 512                      # PE/PSUM block
NCONST = 2 * G + NCHAIN

_compiled = None


def _build():
    import concourse.tile as tile
    from concourse import bacc, mybir
    from concourse.mybir import AluOpType

    nc = bacc.Bacc("TRN2", target_bir_lowering=False, debug=False,
                   num_devices=N_CORES)
    f32 = mybir.dt.float32
    i8 = mybir.dt.int8
    bf16 = mybir.dt.bfloat16

    d_ap = nc.dram_tensor("dd", [NCHAIN, P, J], i8, kind="ExternalInput").ap()
    e_ap = nc.dram_tensor("de", [NCHAIN, P, J], bf16,
                          kind="ExternalInput").ap()
    c_ap = nc.dram_tensor("consts", [P, NCONST], f32,
                          kind="ExternalInput").ap()
    m_ap = nc.dram_tensor("mm", [P, (G + 1) * P], bf16,
                          kind="ExternalInput").ap()
    y_ap = nc.dram_tensor("y", [NCHAIN, P, T], i8, kind="ExternalOutput").ap()

    with tile.TileContext(nc) as tc:
        with (
            tc.tile_pool(name="const", bufs=1) as cpool,
            tc.tile_pool(name="din", bufs=4) as dpool,
            tc.tile_pool(name="ein", bufs=4) as epool,
            tc.tile_pool(name="st", bufs=4) as spool,
            tc.tile_pool(name="yout", bufs=4) as ypool,
            tc.tile_pool(name="ps", bufs=6, space="PSUM") as pspool,
        ):
            dts, ets, yts, sts = {}, {}, {}, {}
            for c in range(NCHAIN):
                dt = dpool.tile([P, J], i8, tag="d")
                dts[c] = dt
                et = epool.tile([P, J], bf16, tag="e")
                ets[c] = et
                yt = ypool.tile([P, T], i8, tag="y")
                yts[c] = yt
                stt = spool.tile([P, J + 1], bf16, tag="st")
                sts[c] = stt

            nc.sync.dma_start(dts[0][:, 0:1024], d_ap[0][:, 0:1024])
            # consts ride the ACT HWDGE ring: the SP ring streams the x
            # inputs back-to-back without per-issue gaps ahead of them
            c_t = cpool.tile([P, NCONST], f32)
            nc.scalar.dma_start(c_t[:], c_ap[:])
            m_t = cpool.tile([P, (G + 1) * P], bf16)
            nc.scalar.dma_start(m_t[:], m_ap[:])
            nc.sync.dma_start(dts[0][:, 1024:], d_ap[0][:, 1024:])
            for c in range(1, NCHAIN):
                nc.sync.dma_start(dts[c][:], d_ap[c])
            for c in range(NCHAIN):
                nc.sync.dma_start(ets[c][:], e_ap[c])
            a2_t = c_t[:, 0:G]
            gv_t = c_t[:, G:2 * G]
            y0_t = c_t[:, 2 * G:]
            ident = m_t[:, G * P:]

            # chain initial state into the shift column of each st tile
            for c in range(NCHAIN):
                nc.scalar.copy(sts[c][:, 0:1], y0_t[:, c:c + 1])

            carries = {c: y0_t[:, c:c + 1] for c in range(NCHAIN)}

            def scan_block(c, j0, n):
                g = c % G
                nc.vector.tensor_tensor_scan(
                    sts[c][:, 1 + j0:1 + j0 + n],
                    a2_t[:, g:g + 1].broadcast_to([P, n]),
                    dts[c][:, j0:j0 + n],
                    initial=carries[c],
                    op0=AluOpType.mult,
                    op1=AluOpType.add,
                )
                carries[c] = sts[c][:, j0 + n:j0 + n + 1]

            def resc_block(c, j0, n, on_act=False):
                g = c % G
                yv = yts[c][:].rearrange("p (j two) -> p j two", two=2)
                if on_act:
                    # ACT is idle early; offloading the first rescales
                    # shortens the packed DVE stream that gates the tail
                    nc.scalar.mul(yv[:, j0:j0 + n, 1],
                                  sts[c][:, 1 + j0:1 + j0 + n],
                                  gv_t[:, g:g + 1])
                else:
                    nc.vector.tensor_scalar(
                        yv[:, j0:j0 + n, 1], sts[c][:, 1 + j0:1 + j0 + n],
                        gv_t[:, g:g + 1], None, AluOpType.mult)

            def pe_block(c, j0, egress_dve=False):
                g = c % G
                ps = pspool.tile([P, PB], f32, tag="ps")
                nc.tensor.matmul(ps[:], m_t[:, g * P:(g + 1) * P],
                                 sts[c][:, j0:j0 + PB],
                                 start=True, stop=False)
                nc.tensor.matmul(ps[:], ident,
                                 ets[c][:, j0:j0 + PB],
                                 start=False, stop=True)
                yv = yts[c][:].rearrange("p (j two) -> p j two", two=2)
                if egress_dve:
                    # drain the last PSUM blocks on DVE once its scan
                    # stream is done - halves the serialized egress tail
                    nc.vector.tensor_scalar(
                        yv[:, j0:j0 + PB, 0], ps[:], 1.0, None,
                        AluOpType.mult)
                else:
                    nc.scalar.copy(yv[:, j0:j0 + PB, 0], ps[:])

            for pair in ((0, 1), (2, 3)):
                for rnd in range(2):
                    if pair == (0, 1) and rnd == 0:
                        scan_block(0, 0, 1024)
                        scan_block(0, 1024, 1024)
                        scan_block(1, 0, SB)
                    else:
                        for c in pair:
                            scan_block(c, rnd * SB, SB)
                    # chain 3 first in the final round: its egress stays on
                    # ACT while chain 2's drains on the freed-up DVE
                    last = pair == (2, 3) and rnd == 1
                    chains = (3, 2) if last else pair
                    for c in chains:
                        resc_block(c, rnd * SB, SB,
                                   on_act=(pair == (0, 1) and rnd == 0))
                        for k in range(SB // PB):
                            j0 = rnd * SB + k * PB
                            pe_block(c, j0, egress_dve=(last and c == 2))
                            if rnd == 1 and j0 + PB == 3 * J // 4:
                                nc.sync.dma_start(
                                    y_ap[c][:, T // 2:3 * T // 4],
                                    yts[c][:, T // 2:3 * T // 4])
                    for c in pair:
                        if rnd == 0:
                            # SP ring is idle once inputs land; keep the ACT
                            # ring's issue pipeline clear for its compute
                            nc.sync.dma_start(
                                y_ap[c][:, :T // 2], yts[c][:, :T // 2])
                        else:
                            nc.sync.dma_start(
                                y_ap[c][:, 3 * T // 4:],
                                yts[c][:, 3 * T // 4:])

    nc.compile()
    return nc


def _get_compiled():
    global _compiled
    if _compiled is None:
        _compiled = _build()
    return _compiled


def _in_maps(inputs, initial_state, smooth):
    import ml_dtypes

    inputs = np.ascontiguousarray(inputs, dtype=np.float32)
    initial_state = np.ascontiguousarray(initial_state, dtype=np.float32)
    smooth = np.ascontiguousarray(smooth, dtype=np.float32)

    w = np.clip(smooth, 0.0, 1.0)
    a = (1.0 - w).astype(np.float32)

    S = float(np.abs(inputs).max()) / 126.5
    Sy = max(float(np.abs(inputs).max()),
             float(np.abs(initial_state).max())) / 126.5
    ws = np.where(w > 0.0, w, 1.0)

    # [core, b, j, par, g, p]; t = 2j + par
    xr = inputs.reshape(N_CORES, B_LOC, J, 2, G, P)
    x_ev = xr[:, :, :, 0]                     # t = 2j   [core, b, j, g, p]
    x_od = xr[:, :, :, 1]                     # t = 2j+1

    D = np.round((a.reshape(G, P) * x_ev + x_od)
                 / (S * (1.0 + a.reshape(G, P)))).astype(np.int8)
    dq = D.transpose(0, 1, 3, 4, 2).reshape(N_CORES, NCHAIN, P, J)

    dm = (w.reshape(G, P) * x_ev / Sy).astype(ml_dtypes.bfloat16)
    de = dm.transpose(0, 1, 3, 4, 2).reshape(N_CORES, NCHAIN, P, J)

    aG = a.reshape(G, P).T                    # [P, G]
    wG = w.reshape(G, P).T
    a2_pg = (aG * aG).astype(np.float32)
    gv_pg = (wG * S * (1.0 + aG) / Sy).astype(np.float32)
    init_all = initial_state / (ws * S * (1.0 + a))

    m_ch = (aG * wG * S * (1.0 + aG) / Sy).astype(np.float32)  # [P, G]
    mm = np.zeros((P, (G + 1) * P), dtype=ml_dtypes.bfloat16)
    for g in range(G):
        mm[:, g * P:(g + 1) * P][np.arange(P), np.arange(P)] = \
            m_ch[:, g].astype(ml_dtypes.bfloat16)
    mm[:, G * P:][np.arange(P), np.arange(P)] = 1.0

    in_maps = []
    for core in range(N_CORES):
        consts = np.empty((P, NCONST), dtype=np.float32)
        consts[:, 0:G] = a2_pg
        consts[:, G:2 * G] = gv_pg
        for b in range(B_LOC):
            for g in range(G):
                consts[:, 2 * G + b * G + g] = init_all[core * B_LOC + b,
                                                        g * P:(g + 1) * P]
        in_maps.append({
            "dd": np.ascontiguousarray(dq[core]),
            "de": np.ascontiguousarray(de[core]),
            "consts": consts,
            "mm": mm,
        })
    return in_maps, S, Sy, w


def kernel(inputs, initial_state, smooth):
    from concourse.bass_utils import run_bass_kernel_spmd

    nc = _get_compiled()
    in_maps, S, Sy, w = _in_maps(inputs, initial_state, smooth)
    res = run_bass_kernel_spmd(nc, in_maps, list(range(N_CORES)))

    yh = np.stack([res.results[c]["y"] for c in range(N_CORES)])
    out = (
        (yh.astype(np.float32) * Sy)
        .reshape(N_CORES, B_LOC, G, P, T)
        .transpose(0, 1, 4, 2, 3)
        .reshape(B, T, C)
    )
    zero = np.clip(np.asarray(smooth, dtype=np.float32), 0.0, 1.0) == 0.0
    if zero.any():
        out[:, :, zero] = np.asarray(initial_state,
                                     dtype=np.float32)[:, None, zero]
    return out


# revision 7
# speedup vs baseline: 1.0869x; 1.0026x over previous
"""EMA linear-recurrence kernel for TRN2, 8 cores. Even/odd PE-hybrid.

Splits each time chain into scan positions t=2j+1 (DVE scan over the
squared-coefficient recurrence st_j = a^2 st_{j-1} + D_j, HALF the columns)
and reconstruction positions t=2j, computed on the otherwise-idle PE as
  psum = diag(a*w*S*(1+a)/S_y) @ st_shifted + I @ d      (bf16, one-shot)
where st_shifted is the scan output tile with the chain initial state in
column 0. Scan-half outputs are rescaled on DVE via the 2x-ported
tensor_scalar (0.5 cyc/col); PE results leave PSUM through ACT copy-convert.
Both int8 output halves interleave into one [P, T] tile via stride-2 views,
so a single contiguous DMA per half ships them out.

Input quantization (host): D_j = round((a x_{2j} + x_{2j+1}) / (S (1+a)))
int8 - the (1+a) normalization keeps the recurrence-propagated quantization
error at S/2 in output units for every w. d = w x_{2j}/S_y in bf16 (one-shot,
no recurrence amplification). a and a^2 stay f32.
"""

import sys

sys.path.insert(0, "/opt/trn_rl_repo")

import numpy as np

B, T, C = 16, 8192, 256
N_CORES = 8
B_LOC = B // N_CORES
P = 128
G = C // P
NCHAIN = B_LOC * G
J = T // 2                    # 4096 scan positions per chain
SB = 2048                     # scan/rescale block (2 per chain)
PB = 512                      # PE/PSUM block
NCONST = 2 * G + NCHAIN

_compiled = None


def _build():
    import concourse.tile as tile
    from concourse import bacc, mybir
    from concourse.mybir import AluOpType

    nc = bacc.Bacc("TRN2", target_bir_lowering=False, debug=False,
                   num_devices=N_CORES)
    f32 = mybir.dt.float32
    i8 = mybir.dt.int8
    bf16 = mybir.dt.bfloat16

    d_ap = nc.dram_tensor("dd", [NCHAIN, P, J], i8, kind="ExternalInput").ap()
    e_ap = nc.dram_tensor("de", [NCHAIN, P, J], bf16,
                          kind="ExternalInput").ap()
    c_ap = nc.dram_tensor("consts", [P, NCONST], f32,
                          kind="ExternalInput").ap()
    m_ap = nc.dram_tensor("mm", [P, (G + 1) * P], bf16,
                          kind="ExternalInput").ap()
    y_ap = nc.dram_tensor("y", [NCHAIN, P, T], i8, kind="ExternalOutput").ap()

    with tile.TileContext(nc) as tc:
        with (
            tc.tile_pool(name="const", bufs=1) as cpool,
            tc.tile_pool(name="din", bufs=4) as dpool,
            tc.tile_pool(name="ein", bufs=4) as epool,
            tc.tile_pool(name="st", bufs=4) as spool,
            tc.tile_pool(name="yout", bufs=4) as ypool,
            tc.tile_pool(name="ps", bufs=6, space="PSUM") as pspool,
        ):
            dts, ets, yts, sts = {}, {}, {}, {}
            for c in range(NCHAIN):
                dt = dpool.tile([P, J], i8, tag="d")
                dts[c] = dt
                et = epool.tile([P, J], bf16, tag="e")
                ets[c] = et
                yt = ypool.tile([P, T], i8, tag="y")
                yts[c] = yt
                stt = spool.tile([P, J + 1], bf16, tag="st")
                sts[c] = stt

            nc.sync.dma_start(dts[0][:, 0:1024], d_ap[0][:, 0:1024])
            # consts ride the ACT HWDGE ring: the SP ring streams the x
            # inputs back-to-back without per-issue gaps ahead of them
            c_t = cpool.tile([P, NCONST], f32)
            nc.scalar.dma_start(c_t[:], c_ap[:])
            m_t = cpool.tile([P, (G + 1) * P], bf16)
            nc.scalar.dma_start(m_t[:], m_ap[:])
            nc.sync.dma_start(dts[0][:, 1024:], d_ap[0][:, 1024:])
            for c in range(1, NCHAIN):
                nc.sync.dma_start(dts[c][:], d_ap[c])
            for c in range(NCHAIN):
                nc.sync.dma_start(ets[c][:], e_ap[c])
            a2_t = c_t[:, 0:G]
            gv_t = c_t[:, G:2 * G]
            y0_t = c_t[:, 2 * G:]
            ident = m_t[:, G * P:]

            # chain initial state into the shift column of each st tile
            for c in range(NCHAIN):
                nc.scalar.copy(sts[c][:, 0:1], y0_t[:, c:c + 1])

            carries = {c: y0_t[:, c:c + 1] for c in range(NCHAIN)}

            def scan_block(c, j0, n):
                g = c % G
                nc.vector.tensor_tensor_scan(
                    sts[c][:, 1 + j0:1 + j0 + n],
                    a2_t[:, g:g + 1].broadcast_to([P, n]),
                    dts[c][:, j0:j0 + n],
                    initial=carries[c],
                    op0=AluOpType.mult,
                    op1=AluOpType.add,
                )
                carries[c] = sts[c][:, j0 + n:j0 + n + 1]

            def resc_block(c, j0, n, on_act=False):
                g = c % G
                yv = yts[c][:].rearrange("p (j two) -> p j two", two=2)
                if on_act:
                    # ACT is idle early; offloading the first rescales
                    # shortens the packed DVE stream that gates the tail
                    nc.scalar.mul(yv[:, j0:j0 + n, 1],
                                  sts[c][:, 1 + j0:1 + j0 + n],
                                  gv_t[:, g:g + 1])
                else:
                    nc.vector.tensor_scalar(
                        yv[:, j0:j0 + n, 1], sts[c][:, 1 + j0:1 + j0 + n],
                        gv_t[:, g:g + 1], None, AluOpType.mult)

            def pe_block(c, j0, egress_dve=False):
                g = c % G
                ps = pspool.tile([P, PB], f32, tag="ps")
                nc.tensor.matmul(ps[:], m_t[:, g * P:(g + 1) * P],
                                 sts[c][:, j0:j0 + PB],
                                 start=True, stop=False)
                nc.tensor.matmul(ps[:], ident,
                                 ets[c][:, j0:j0 + PB],
                                 start=False, stop=True)
                yv = yts[c][:].rearrange("p (j two) -> p j two", two=2)
                if egress_dve:
                    # drain the last PSUM blocks on DVE once its scan
                    # stream is done - halves the serialized egress tail
                    nc.vector.tensor_scalar(
                        yv[:, j0:j0 + PB, 0], ps[:], 1.0, None,
                        AluOpType.mult)
                else:
                    nc.scalar.copy(yv[:, j0:j0 + PB, 0], ps[:])

            for pair in ((0, 1), (2, 3)):
                for rnd in range(2):
                    if pair == (0, 1) and rnd == 0:
                        scan_block(0, 0, 1024)
                        scan_block(0, 1024, 1024)
                        scan_block(1, 0, SB)
                    else:
                        for c in pair:
                            scan_block(c, rnd * SB, SB)
                    # chain 3 first in the final round: its egress stays on
                    # ACT while chain 2's drains on the freed-up DVE
                    last = pair == (2, 3) and rnd == 1
                    chains = (3, 2) if last else pair
                    for c in chains:
                        resc_block(c, rnd * SB, SB,
                                   on_act=(pair == (0, 1) and rnd == 0))
                        for k in range(SB // PB):
                            j0 = rnd * SB + k * PB
                            # ACT finishes its tail ~1us before DVE now:
                            # hand it back one of chain 2's egress blocks
                            pe_block(c, j0,
                                     egress_dve=(last and c == 2 and k < 3))
                            if rnd == 1 and j0 + PB == 3 * J // 4:
                                nc.sync.dma_start(
                                    y_ap[c][:, T // 2:3 * T // 4],
                                    yts[c][:, T // 2:3 * T // 4])
                    for c in pair:
                        if rnd == 0:
                            # SP ring is idle once inputs land; keep the ACT
                            # ring's issue pipeline clear for its compute
                            nc.sync.dma_start(
                                y_ap[c][:, :T // 2], yts[c][:, :T // 2])
                        else:
                            nc.sync.dma_start(
                                y_ap[c][:, 3 * T // 4:],
                                yts[c][:, 3 * T // 4:])

    nc.compile()
    return nc


def _get_compiled():
    global _compiled
    if _compiled is None:
        _compiled = _build()
    return _compiled


def _in_maps(inputs, initial_state, smooth):
    import ml_dtypes

    inputs = np.ascontiguousarray(inputs, dtype=np.float32)
    initial_state = np.ascontiguousarray(initial_state, dtype=np.float32)
    smooth = np.ascontiguousarray(smooth, dtype=np.float32)

    w = np.clip(smooth, 0.0, 1.0)
    a = (1.0 - w).astype(np.float32)

    S = float(np.abs(inputs).max()) / 126.5
    Sy = max(float(np.abs(inputs).max()),
             float(np.abs(initial_state).max())) / 126.5
    ws = np.where(w > 0.0, w, 1.0)

    # [core, b, j, par, g, p]; t = 2j + par
    xr = inputs.reshape(N_CORES, B_LOC, J, 2, G, P)
    x_ev = xr[:, :, :, 0]                     # t = 2j   [core, b, j, g, p]
    x_od = xr[:, :, :, 1]                     # t = 2j+1

    D = np.round((a.reshape(G, P) * x_ev + x_od)
                 / (S * (1.0 + a.reshape(G, P)))).astype(np.int8)
    dq = D.transpose(0, 1, 3, 4, 2).reshape(N_CORES, NCHAIN, P, J)

    dm = (w.reshape(G, P) * x_ev / Sy).astype(ml_dtypes.bfloat16)
    de = dm.transpose(0, 1, 3, 4, 2).reshape(N_CORES, NCHAIN, P, J)

    aG = a.reshape(G, P).T                    # [P, G]
    wG = w.reshape(G, P).T
    a2_pg = (aG * aG).astype(np.float32)
    gv_pg = (wG * S * (1.0 + aG) / Sy).astype(np.float32)
    init_all = initial_state / (ws * S * (1.0 + a))

    m_ch = (aG * wG * S * (1.0 + aG) / Sy).astype(np.float32)  # [P, G]
    mm = np.zeros((P, (G + 1) * P), dtype=ml_dtypes.bfloat16)
    for g in range(G):
        mm[:, g * P:(g + 1) * P][np.arange(P), np.arange(P)] = \
            m_ch[:, g].astype(ml_dtypes.bfloat16)
    mm[:, G * P:][np.arange(P), np.arange(P)] = 1.0

    in_maps = []
    for core in range(N_CORES):
        consts = np.empty((P, NCONST), dtype=np.float32)
        consts[:, 0:G] = a2_pg
        consts[:, G:2 * G] = gv_pg
        for b in range(B_LOC):
            for g in range(G):
                consts[:, 2 * G + b * G + g] = init_all[core * B_LOC + b,
                                                        g * P:(g + 1) * P]
        in_maps.append({
            "dd": np.ascontiguousarray(dq[core]),
            "de": np.ascontiguousarray(de[core]),
            "consts": consts,
            "mm": mm,
        })
    return in_maps, S, Sy, w


def kernel(inputs, initial_state, smooth):
    from concourse.bass_utils import run_bass_kernel_spmd

    nc = _get_compiled()
    in_maps, S, Sy, w = _in_maps(inputs, initial_state, smooth)
    res = run_bass_kernel_spmd(nc, in_maps, list(range(N_CORES)))

    yh = np.stack([res.results[c]["y"] for c in range(N_CORES)])
    out = (
        (yh.astype(np.float32) * Sy)
        .reshape(N_CORES, B_LOC, G, P, T)
        .transpose(0, 1, 4, 2, 3)
        .reshape(B, T, C)
    )
    zero = np.clip(np.asarray(smooth, dtype=np.float32), 0.0, 1.0) == 0.0
    if zero.any():
        out[:, :, zero] = np.asarray(initial_state,
                                     dtype=np.float32)[:, None, zero]
    return out
